# revision 32
# baseline (speedup 1.0000x reference)
"""Decoder-only attention kernel for Trainium2 (8 NeuronCores).

Sharding: tensor-parallel over heads (4 groups of 4 heads) x data-parallel
over batch (2), = 8 cores. Each core computes, for its batch b and its 4
heads, the partial output  sum_h (softmax(causal(q_h k_h^T / 8)) v_h) @ O_h
as a [T, D] array; the host sums the 4 partials per batch and adds Ob.

v3 design ("stall-free pipeline"), on top of v2's bf16 + interleaved-fill
transposed flash attention:
  - v2 recap: xT [D,T] resident bf16; qT/kT [dk,T] with head pairs at
    partitions 0/64 (the two score matmuls of a pair run CONCURRENTLY in
    disjoint PE row-groups — verified in the NTFF trace, dstart ~3ns);
    scoresT[s,tq] per head; causal diag masked ON PE via one accumulating
    masku^T @ I matmul; softmax denominator rides the z matmul as a ones
    column of v (M=65); single merged unit loop (s-chunk j, head pair)
    with projection/O-proj closures interleaved as PE fill.
  - v3 fixes, all trace-driven (baseline 236us):
    * the norm chain used nc.vector.reciprocal on [1,512]: 3.3us each on
      DVE. 16 of them = 53us, and the chain blocked the next pair's z
      matmuls (pz pool slot) for 4-5us -> 9 PE idle gaps (41us) -> HAM
      re-throttled to K=4/8 every time (88us at half clock). Now uses
      reciprocal_approx_fast (~0.7us, 18-bit accurate - plenty above the
      bf16 noise floor).
    * q/k bias+eviction moved from DVE tensor_scalar_add to the Scalar
      engine as an Identity activation with per-partition bias AP (exp
      and identity share one act table -> no table reload). -12us DVE.
    * output is bf16 (rounding ~2^-9 relative, well inside tolerance):
      halves the 8MB output DMA; the two [128,512] O-proj evictions per
      128-row chunk merge into one [128,1024] SBUF tile and ONE DMA,
      issued from gpsimd (software DGE, ~25ns engine cost) - the v2 tail
      was 11.8us of serial 610ns DMA issues on the Sync engine.
    * input DMAs: xT block 0 lands as ONE strided DMA (was 8 x 610ns
      serial issues; first matmul moved ~12.7us -> ~5us), small tensors
      (biases, masks) issue from gpsimd, weights stay on sync.
    * act table preloaded with a dummy exp during the DMA head, so the
      first real exp doesn't pay the 1.3us ACT_TABLE_LOAD.
    * all-zero V bias detected at runtime -> the 16 ones-row bias
      matmuls are compiled out (the general path stays available).
  - PSUM budget (8 banks): scores ps x2 [128,1024] (4), z accum pz x2
    [65,512] (2), fill pool x2 [128,512] (2).
"""

import sys

import numpy as np

if "/opt/trn_rl_repo" not in sys.path:
    sys.path.insert(0, "/opt/trn_rl_repo")

# Model shapes (hardcoded; kernel.py must be self-contained).
B = 2
T = 2048
D = 1024
NH = 16  # total heads
H = 4  # heads per core
DK = 64
NCORES = 8

TQ = 512  # tq chunk (matmul moving free dim)
SK = 128  # s chunk (partition dim of scoresT)

_PROGRAM_CACHE = {}

# test.py can flip these before calling kernel()
TRACE = False
TRACE_KWARGS = {}


def _build_program(t=T, d=D, reps=1, vbias=True, qkbias=True, debug=False):
    import concourse.bass as bass  # noqa: F401
    import concourse.mybir as mybir
    import concourse.tile as tile
    from concourse import bacc

    f32 = mybir.dt.float32
    bf16 = mybir.dt.bfloat16
    IDENT = mybir.ActivationFunctionType.Identity

    P = 128
    DC = d // P  # d_model chunks (contraction for projections)
    SC = t // SK  # s chunks
    TC = t // TQ  # tq chunks
    VW = H * (DK + 1)  # v block width per s-chunk: 4 heads x (64 v + 1 ones)
    m = H * DK

    nc = bacc.Bacc(
        "TRN2",
        target_bir_lowering=False,
        debug=False,
        enable_asserts=True,
        num_devices=NCORES,
    )

    xT = nc.dram_tensor("xT", [d, t], bf16, kind="ExternalInput").ap()
    wq = nc.dram_tensor("wq", [d, m], bf16, kind="ExternalInput").ap()
    wk = nc.dram_tensor("wk", [d, m], bf16, kind="ExternalInput").ap()
    wv = nc.dram_tensor("wv", [d, m], bf16, kind="ExternalInput").ap()
    wo = nc.dram_tensor("wo", [m, d], bf16, kind="ExternalInput").ap()
    qb = nc.dram_tensor("qb", [P, 2], f32, kind="ExternalInput").ap()
    kb = nc.dram_tensor("kb", [P, 2], f32, kind="ExternalInput").ap()
    vb = nc.dram_tensor("vb", [1, m], bf16, kind="ExternalInput").ap()
    masku = nc.dram_tensor("masku", [P, P], bf16, kind="ExternalInput").ap()
    ident = nc.dram_tensor("ident", [P, P], bf16, kind="ExternalInput").ap()
    out = nc.dram_tensor("out", [t, d], bf16, kind="ExternalOutput").ap()
    if debug:
        SC_ = t // SK
        VW_ = H * (DK + 1)
        dbg_xT = nc.dram_tensor("dbg_xT", [P, (d // P) * t], bf16, kind="ExternalOutput").ap()
        dbg_qT = nc.dram_tensor("dbg_qT", [P, t], bf16, kind="ExternalOutput").ap()
        dbg_kT = nc.dram_tensor("dbg_kT", [P, t], bf16, kind="ExternalOutput").ap()
        dbg_v = nc.dram_tensor("dbg_v", [P, SC_ * VW_], bf16, kind="ExternalOutput").ap()
        dbg_rcp = nc.dram_tensor("dbg_rcp", [16, TQ], f32, kind="ExternalOutput").ap()
        dbg_msk = nc.dram_tensor("dbg_msk", [P, 2 * P], bf16, kind="ExternalOutput").ap()

    with tile.TileContext(nc) as tc:
        from contextlib import ExitStack

        ctx = ExitStack()
        with ctx:
            const = ctx.enter_context(tc.tile_pool(name="const", bufs=1))

            # ---- resident SBUF tensors ----
            # xT as ONE tile so block-0 can land via a single strided DMA
            xT_all = const.tile([P, DC * t], bf16, tag="xT")
            xT_sb = [xT_all[:, c * t : (c + 1) * t] for c in range(DC)]
            wq_sb = const.tile([P, DC * m], bf16, tag="wq")
            wk_sb = const.tile([P, DC * m], bf16, tag="wk")
            wv_sb = const.tile([P, DC * m], bf16, tag="wv")
            wo_sb = const.tile([P, 2 * d], bf16, tag="wo")
            qb_sb = const.tile([P, 2], f32, tag="qb")
            kb_sb = const.tile([P, 2], f32, tag="kb")
            vb_sb = const.tile([1, m], bf16, tag="vb")
            masku_sb = const.tile([P, P], bf16, tag="masku")
            ident_sb = const.tile([P, P], bf16, tag="ident")
            ones_row = const.tile([1, P], bf16, tag="ones_row")
            v_sb = const.tile([P, SC * VW], bf16, tag="v")
            qT_sb = [
                const.tile([P, t], bf16, tag=f"qT{p}", name=f"qT{p}") for p in range(2)
            ]
            kT_sb = [
                const.tile([P, t], bf16, tag=f"kT{p}", name=f"kT{p}") for p in range(2)
            ]
            # one zT tile per (pair, tq-block): separate tiles keep the
            # O-projection of block n free of false deps on later norms
            zT_sb = [
                [
                    const.tile([P, TQ], bf16, tag=f"zT{p}_{tb}", name=f"zT{p}_{tb}")
                    for tb in range(TC)
                ]
                for p in range(2)
            ]

            # ---- input DMAs ----
            # wv + xT block0 first (v-proj of block 0 is the first PE work).
            # Bulk inputs ride TWO queues in parallel (each DMA queue tops
            # out near ~180 GB/s - half the aggregate - so every big tensor
            # splits into c-chunk halves issued from the sync and scalar
            # engines). Order on each queue = need order: wv + x block0
            # gate the first matmuls, then wq/wk, then the rest.
            HC = DC // 2

            def w_dma(dst_sb, src):
                dst = dst_sb.rearrange("p (c m) -> p c m", c=DC)
                srcr = src.rearrange("(c p) m -> p c m", p=P)
                nc.sync.dma_start(dst[:, 0:HC], srcr[:, 0:HC])
                nc.scalar.dma_start(dst[:, HC:DC], srcr[:, HC:DC])

            def x_dma(lo, hi):
                dst = xT_all.rearrange("p (c t) -> p c t", c=DC)
                srcr = xT.rearrange("(c p) t -> p c t", p=P)
                nc.sync.dma_start(dst[:, 0:HC, lo:hi], srcr[:, 0:HC, lo:hi])
                nc.scalar.dma_start(dst[:, HC:DC, lo:hi], srcr[:, HC:DC, lo:hi])

            w_dma(wv_sb, wv)
            x_dma(0, TQ)
            nc.gpsimd.dma_start(vb_sb[:], vb[:])
            nc.gpsimd.dma_start(qb_sb[:], qb[:])
            nc.gpsimd.dma_start(kb_sb[:], kb[:])
            nc.gpsimd.dma_start(masku_sb[:], masku[:])
            nc.gpsimd.dma_start(ident_sb[:], ident[:])
            w_dma(wq_sb, wq)
            w_dma(wk_sb, wk)
            if TC > 1:
                x_dma(TQ, 2 * TQ)
            wo_r = wo_sb.rearrange("p (c d2) -> p c d2", c=2)
            wo_src = wo.rearrange("(c p) d2 -> p c d2", p=P)
            nc.sync.dma_start(wo_r[:, 0:1], wo_src[:, 0:1])
            nc.scalar.dma_start(wo_r[:, 1:2], wo_src[:, 1:2])
            x_dma(2 * TQ, t)

            ones_stage = const.tile([P, 1], bf16, tag="ones_stage")
            nc.vector.memset(ones_stage[:], 1.0)
            # preload the exp act table while DMAs stream (else the first
            # real exp pays ~1.3us ACT_TABLE_LOAD on the critical path)
            warm_act = const.tile([1, 1], f32, tag="warm_act")
            nc.scalar.activation(
                warm_act[:],
                ones_stage[0:1, 0:1],
                mybir.ActivationFunctionType.Exp,
                scale=1.0,
            )
            nc.vector.tensor_copy(ones_row[:], ones_stage[0:1, :].to_broadcast([1, P]))
            # ones columns of the v blocks (col 64 of each head's 65-wide slot)
            nc.vector.tensor_copy(
                v_sb.rearrange("p (j h c) -> p j h c", h=H, c=DK + 1)[:, :, :, DK :],
                ones_stage[:, None, None, :].to_broadcast([P, SC, H, 1]),
            )

            def _phases():
                with (
                    tc.tile_pool(name="ps", bufs=2, space="PSUM") as ps_pool,
                    tc.tile_pool(name="pz", bufs=2, space="PSUM") as pz_pool,
                    tc.tile_pool(name="pfill", bufs=2, space="PSUM") as pf_pool,
                    tc.tile_pool(name="expt", bufs=6) as exp_pool,
                    tc.tile_pool(name="rcp", bufs=4) as rcp_pool,
                    tc.tile_pool(name="rbs", bufs=4) as rbs_pool,
                    tc.tile_pool(name="osb", bufs=4) as out_pool,
                ):
                    # NOTE: no PE warm-up matmuls here. They keep the HAM at
                    # 8/8 through the DMA head, but with all 8 cores running
                    # them in lockstep the CHIP power limiter kicks in (P0,
                    # PE ~2.0 GHz) and the whole run slows ~20% - measured
                    # 212us vs 176us. Power is the scarcer resource.
                    # ---------- fill closures: projections ----------
                    # each closure owns its PSUM tile's FULL lifecycle
                    # (alloc -> matmuls -> evict) so pool buffers never hold
                    # half-accumulated state across other closures.
                    def proj_closures(tp):
                        cls = []
                        sl = slice(tp * TQ, (tp + 1) * TQ)

                        def v_mm(i):
                            def go():
                                pv = pf_pool.tile([P, TQ], f32, tag="pf", name="pv")
                                for c in range(DC):
                                    nc.tensor.matmul(
                                        pv[:, 0:m],
                                        xT_sb[c][:, i * P : (i + 1) * P],
                                        wv_sb[:, c * m : (c + 1) * m],
                                        start=(c == 0),
                                        stop=(c == DC - 1) and not vbias,
                                    )
                                if vbias:
                                    nc.tensor.matmul(
                                        pv[:, 0:m],
                                        ones_row[0:1, :],
                                        vb_sb[0:1, :],
                                        start=False,
                                        stop=True,
                                    )
                                nc.vector.tensor_copy(
                                    v_sb.rearrange(
                                        "p (j h c) -> p j h c", h=H, c=DK + 1
                                    )[:, i, :, 0:DK],
                                    pv[:, 0:m].rearrange("p (h c) -> p h c", c=DK),
                                )

                            return go

                        def qk_mm(p, which):
                            w_sb = wq_sb if which == 0 else wk_sb
                            b_sb = qb_sb if which == 0 else kb_sb
                            dst = qT_sb[p] if which == 0 else kT_sb[p]

                            def go():
                                pq = pf_pool.tile([P, TQ], f32, tag="pf", name="pq")
                                for c in range(DC):
                                    nc.tensor.matmul(
                                        pq[:],
                                        w_sb[:, c * m + p * P : c * m + (p + 1) * P],
                                        xT_sb[c][:, sl],
                                        start=(c == 0),
                                        stop=(c == DC - 1),
                                    )
                                # bias + eviction fused on the Scalar engine
                                # (identity shares the exp act table, and the
                                # DVE FIFO stays clear for the norm chains)
                                nc.scalar.activation(
                                    dst[:, sl],
                                    pq[:],
                                    IDENT,
                                    bias=b_sb[:, p : p + 1],
                                    scale=1.0,
                                )

                            return go

                        for i in range(4 * tp, 4 * tp + 4):
                            cls.append(v_mm(i))
                        for p in range(2):
                            for which in range(2):
                                cls.append(qk_mm(p, which))
                        return cls

                    # ---------- fill closures: O projection ----------
                    # one closure per 128-row output chunk: both d-halves'
                    # matmuls, two casts into one [128, d] bf16 tile, ONE
                    # output DMA issued from gpsimd (software DGE).
                    def o_proj_closures(tq_c, act_evict_from=4, evict_mode=1):
                        cls = []

                        def o_mm(i, act_evict):
                            def go():
                                ic = i - 4 * tq_c
                                ot = out_pool.tile([P, d], bf16, tag="osb", name="ot")
                                for d2 in range(d // TQ):
                                    po = pf_pool.tile(
                                        [P, TQ], f32, tag="pf", name="po"
                                    )
                                    for kc in range(2):
                                        nc.tensor.matmul(
                                            po[:],
                                            zT_sb[kc][tq_c][
                                                :, ic * P : (ic + 1) * P
                                            ],
                                            wo_sb[
                                                :,
                                                kc * d
                                                + d2 * TQ : kc * d
                                                + (d2 + 1) * TQ,
                                            ],
                                            start=(kc == 0),
                                            stop=(kc == 1),
                                        )
                                    # eviction engine: 0 = DVE (mid-block),
                                    # 1 = Scalar (block tails - DVE FIFO is
                                    # busy with norm chains), 2 = split DVE/
                                    # Scalar in parallel (kernel tail, both
                                    # engines idle)
                                    dst = ot[:, d2 * TQ : (d2 + 1) * TQ]
                                    if act_evict == 1 or (
                                        act_evict == 2 and d2 == 1
                                    ):
                                        nc.scalar.activation(
                                            dst,
                                            po[:],
                                            mybir.ActivationFunctionType.Copy,
                                        )
                                    else:
                                        nc.vector.tensor_copy(dst, po[:])
                                nc.sync.dma_start(
                                    out[i * P : (i + 1) * P, :], ot[:]
                                )

                            return go

                        for n, i in enumerate(range(4 * tq_c, 4 * tq_c + 4)):
                            cls.append(o_mm(i, evict_mode if n >= act_evict_from else 0))
                        return cls

                    # ---------- attention for one tq block ----------
                    # one head-PAIR (p) at a time: per s-chunk j both heads'
                    # scores land in one 2-bank ps tile and get a single
                    # fused [128,1024] exp (fewer Act instructions).
                    def attention(tcq, fills):
                        sl = slice(tcq * TQ, (tcq + 1) * TQ)
                        nsc = 4 * tcq + 4  # s chunks in causal range
                        fill_rate = len(fills) / max(1, 2 * nsc)
                        fill_acc = [0.0]

                        def pair_loop(p):
                            pz = [
                                pz_pool.tile([DK + 1, TQ], f32, tag="pz", name="pz")
                                for _ in range(2)
                            ]

                            def scores(j):
                                ps = ps_pool.tile(
                                    [P, 2 * TQ], f32, tag="ps", name="ps"
                                )
                                rdiag = j - 4 * tcq
                                for hh in range(2):
                                    o = hh * TQ
                                    nc.tensor.matmul(
                                        ps[:, o : o + TQ],
                                        kT_sb[p][
                                            hh * DK : (hh + 1) * DK,
                                            j * SK : (j + 1) * SK,
                                        ],
                                        qT_sb[p][hh * DK : (hh + 1) * DK, sl],
                                        start=True,
                                        stop=(rdiag < 0),
                                        skip_group_check=True,
                                    )
                                    if rdiag >= 0:
                                        # causal mask of the diag 128 block:
                                        # ps += maskU^T @ I (-1e5 above diag)
                                        nc.tensor.matmul(
                                            ps[
                                                :,
                                                o
                                                + rdiag * SK : o
                                                + (rdiag + 1) * SK,
                                            ],
                                            masku_sb[:],
                                            ident_sb[:],
                                            start=False,
                                            stop=True,
                                            skip_group_check=True,
                                        )
                                et = exp_pool.tile(
                                    [P, 2 * TQ], bf16, tag="expt", name="et"
                                )
                                nc.scalar.activation(
                                    et[:],
                                    ps[:],
                                    mybir.ActivationFunctionType.Exp,
                                    scale=0.125,
                                )
                                return et

                            def zmm(j, et):
                                rdiag = j - 4 * tcq
                                for hh in range(2):
                                    o = hh * TQ
                                    h = 2 * p + hh
                                    vap = v_sb[
                                        :,
                                        j * VW
                                        + h * (DK + 1) : j * VW
                                        + (h + 1) * (DK + 1),
                                    ]
                                    if rdiag < 0:
                                        nc.tensor.matmul(
                                            pz[hh][:],
                                            vap,
                                            et[:, o : o + TQ],
                                            start=(j == 0),
                                            stop=False,
                                            skip_group_check=True,
                                        )
                                    else:
                                        nc.tensor.matmul(
                                            pz[hh][
                                                :, rdiag * SK : (rdiag + 1) * SK
                                            ],
                                            vap,
                                            et[
                                                :,
                                                o
                                                + rdiag * SK : o
                                                + (rdiag + 1) * SK,
                                            ],
                                            start=(j == 0),
                                            stop=True,
                                            skip_group_check=True,
                                        )
                                        if rdiag < 3:
                                            nc.tensor.matmul(
                                                pz[hh][:, (rdiag + 1) * SK : TQ],
                                                vap,
                                                et[
                                                    :,
                                                    o
                                                    + (rdiag + 1) * SK : o
                                                    + TQ,
                                                ],
                                                start=(j == 0),
                                                stop=False,
                                                skip_group_check=True,
                                            )

                            # software pipeline: scores run 2 j's ahead of z;
                            # fills are spread evenly over the pop slots so
                            # late units (and the norm window) stay covered
                            ets = {}
                            for u in range(nsc + 2):
                                if u < nsc:
                                    ets[u] = scores(u)
                                if u >= 2:
                                    zmm(u - 2, ets.pop(u - 2))
                                    fill_acc[0] += fill_rate
                                    while fill_acc[0] >= 1.0 and fills:
                                        fills.popleft()()
                                        fill_acc[0] -= 1.0

                            # normalization: zT = pz[0:64] * (1/denom row).
                            # reciprocal_approx_fast is ~5x cheaper than
                            # nc.vector.reciprocal and 18-bit accurate -
                            # far above the bf16 noise floor downstream.
                            for hh in range(2):
                                den = rcp_pool.tile(
                                    [1, TQ], f32, tag="den", name="den"
                                )
                                nc.vector.tensor_copy(den[:], pz[hh][DK : DK + 1, :])
                                rcp = rcp_pool.tile(
                                    [1, TQ], f32, tag="rcp", name="rcp"
                                )
                                nc.vector.reciprocal_approx_fast(rcp[:], den[:])
                                if debug:
                                    ri = 4 * tcq + 2 * p + hh
                                    nc.sync.dma_start(
                                        dbg_rcp[ri : ri + 1, :], rcp[:]
                                    )
                                rb_sb = rbs_pool.tile(
                                    [DK, TQ], f32, tag="rbs", name="rbs"
                                )
                                nc.gpsimd.partition_broadcast(rb_sb[:], rcp[:])
                                nc.vector.tensor_mul(
                                    zT_sb[p][tcq][hh * DK : (hh + 1) * DK, :],
                                    pz[hh][0:DK, :],
                                    rb_sb[:],
                                )

                        for p in range(2):
                            pair_loop(p)

                    # ---------- main schedule ----------
                    from collections import deque

                    # Fill distribution across blocks. Block tcq has
                    # 2*(4*tcq+4) fill slots; later blocks have MORE slots
                    # but no next-block projections left, so the O
                    # projections are deferred toward them (any O-proj of
                    # block b can run during any block > b). Without this,
                    # block 3's 32 units run fill-dry and pace on the Act
                    # engine's exp latency while the PE idles.
                    for cl in proj_closures(0):
                        cl()
                    for tcq in range(TC):
                        fills = deque()
                        reserve = []
                        if tcq + 1 < TC:
                            fills.extend(proj_closures(tcq + 1))
                        if tcq == TC - 2 and TC >= 3:
                            fills.extend(o_proj_closures(0))
                        if tcq == TC - 1:
                            # the last block's trailing norm chain is covered
                            # by reserve closures whose evictions ride the
                            # (then-idle) Scalar engine, not the DVE
                            if TC >= 4:
                                fills.extend(o_proj_closures(TC - 3))
                            ocls = o_proj_closures(TC - 2, act_evict_from=1)
                            fills.extend(ocls[:1])
                            reserve = ocls[1:]
                        attention(tcq, fills)
                        while fills:
                            fills.popleft()()
                        for cl in reserve:
                            cl()
                    for cl in o_proj_closures(
                        TC - 1, act_evict_from=0, evict_mode=2
                    ):
                        cl()
                    if debug:
                        nc.sync.dma_start(dbg_xT[:], xT_all[:])
                        nc.sync.dma_start(dbg_qT[:], qT_sb[0][:])
                        nc.sync.dma_start(dbg_kT[:], kT_sb[0][:])
                        nc.sync.dma_start(dbg_v[:], v_sb[:])
                        nc.sync.dma_start(dbg_msk[:, 0:P], masku_sb[:])
                        nc.sync.dma_start(dbg_msk[:, P : 2 * P], ident_sb[:])

            if reps == 1:
                _phases()
            else:
                with tc.For_i(0, reps, 1):
                    _phases()

    nc.compile()
    return nc


def _get_program(t=T, d=D, reps=1, vbias=True, qkbias=True, debug=False):
    key = (t, d, reps, vbias, qkbias, debug)
    if key not in _PROGRAM_CACHE:
        _PROGRAM_CACHE[key] = _build_program(t, d, reps, vbias, qkbias, debug)
    return _PROGRAM_CACHE[key]


def _bf16(a):
    import ml_dtypes

    return np.ascontiguousarray(a.astype(ml_dtypes.bfloat16))


def _make_masks():
    import ml_dtypes

    i = np.arange(SK)[:, None]
    j = np.arange(SK)[None, :]
    # masku[k, m] = -1e5 where m > k (strictly upper): M = masku^T @ I gives
    # M[s, t] = masku[t, s] = -1e5 where s > t (key after query -> masked)
    masku = np.where(j > i, np.float32(-100000.0), np.float32(0.0))
    ident = np.eye(SK, dtype=np.float32)
    return (
        np.ascontiguousarray(masku.astype(ml_dtypes.bfloat16)),
        np.ascontiguousarray(ident.astype(ml_dtypes.bfloat16)),
    )


def _core_inputs(x, Qs, Qbs, Ks, Kbs, Vs, Vbs, O, b, g, masku, ident):
    hs = slice(H * g, H * (g + 1))
    xT_b = _bf16(np.ascontiguousarray(x[b].T))  # [D, T]
    wq_g = _bf16(Qs[hs].transpose(1, 0, 2).reshape(D, H * DK))
    wk_g = _bf16(Ks[hs].transpose(1, 0, 2).reshape(D, H * DK))
    wv_g = _bf16(Vs[hs].transpose(1, 0, 2).reshape(D, H * DK))
    wo_g = _bf16(O[hs].reshape(H * DK, D))
    qb_flat = Qbs[hs].reshape(H * DK)
    kb_flat = Kbs[hs].reshape(H * DK)
    qb_g = np.ascontiguousarray(np.stack([qb_flat[0:128], qb_flat[128:256]], axis=1))
    kb_g = np.ascontiguousarray(np.stack([kb_flat[0:128], kb_flat[128:256]], axis=1))
    vb_g = _bf16(Vbs[hs].reshape(1, H * DK))
    return {
        "xT": xT_b,
        "wq": wq_g,
        "wk": wk_g,
        "wv": wv_g,
        "wo": wo_g,
        "qb": qb_g,
        "kb": kb_g,
        "vb": vb_g,
        "masku": masku,
        "ident": ident,
    }


def _build_in_maps(x, Qs, Qbs, Ks, Kbs, Vs, Vbs, O_):
    masku, ident = _make_masks()
    in_maps = []
    for core in range(NCORES):
        b, g = divmod(core, NH // H)
        in_maps.append(
            _core_inputs(x, Qs, Qbs, Ks, Kbs, Vs, Vbs, O_, b, g, masku, ident)
        )
    return in_maps


def kernel(normalized_resid_pre, Qs, Qbs, Ks, Kbs, Vs, Vbs, O, Ob):
    from concourse.bass_utils import run_bass_kernel_spmd

    x = np.asarray(normalized_resid_pre, dtype=np.float32)
    Qs, Qbs = np.asarray(Qs, np.float32), np.asarray(Qbs, np.float32)
    Ks, Kbs = np.asarray(Ks, np.float32), np.asarray(Kbs, np.float32)
    Vs, Vbs = np.asarray(Vs, np.float32), np.asarray(Vbs, np.float32)
    O_, Ob = np.asarray(O, np.float32), np.asarray(Ob, np.float32)

    vbias = bool(np.any(Vbs != 0.0))
    qkbias = bool(np.any(Qbs != 0.0) or np.any(Kbs != 0.0))
    nc = _get_program(vbias=vbias, qkbias=qkbias)
    in_maps = _build_in_maps(x, Qs, Qbs, Ks, Kbs, Vs, Vbs, O_)

    res = run_bass_kernel_spmd(
        nc, in_maps, core_ids=list(range(NCORES)), trace=TRACE, **TRACE_KWARGS
    )
    kernel.last_results = res

    out = np.zeros((B, T, D), dtype=np.float32)
    for core in range(NCORES):
        b, g = divmod(core, NH // H)
        out[b] += np.asarray(res.results[core]["out"], dtype=np.float32)
    out += Ob[None, None, :]
    return out


# revision 33
# speedup vs baseline: 1.0080x; 1.0080x over previous
"""Decoder-only attention kernel for Trainium2 (8 NeuronCores).

Sharding: tensor-parallel over heads (4 groups of 4 heads) x data-parallel
over batch (2), = 8 cores. Each core computes, for its batch b and its 4
heads, the partial output  sum_h (softmax(causal(q_h k_h^T / 8)) v_h) @ O_h
as a [T, D] array; the host sums the 4 partials per batch and adds Ob.

v3 design ("stall-free pipeline"), on top of v2's bf16 + interleaved-fill
transposed flash attention:
  - v2 recap: xT [D,T] resident bf16; qT/kT [dk,T] with head pairs at
    partitions 0/64 (the two score matmuls of a pair run CONCURRENTLY in
    disjoint PE row-groups — verified in the NTFF trace, dstart ~3ns);
    scoresT[s,tq] per head; causal diag masked ON PE via one accumulating
    masku^T @ I matmul; softmax denominator rides the z matmul as a ones
    column of v (M=65); single merged unit loop (s-chunk j, head pair)
    with projection/O-proj closures interleaved as PE fill.
  - v3 fixes, all trace-driven (baseline 236us):
    * the norm chain used nc.vector.reciprocal on [1,512]: 3.3us each on
      DVE. 16 of them = 53us, and the chain blocked the next pair's z
      matmuls (pz pool slot) for 4-5us -> 9 PE idle gaps (41us) -> HAM
      re-throttled to K=4/8 every time (88us at half clock). Now uses
      reciprocal_approx_fast (~0.7us, 18-bit accurate - plenty above the
      bf16 noise floor).
    * q/k bias+eviction moved from DVE tensor_scalar_add to the Scalar
      engine as an Identity activation with per-partition bias AP (exp
      and identity share one act table -> no table reload). -12us DVE.
    * output is bf16 (rounding ~2^-9 relative, well inside tolerance):
      halves the 8MB output DMA; the two [128,512] O-proj evictions per
      128-row chunk merge into one [128,1024] SBUF tile and ONE DMA,
      issued from gpsimd (software DGE, ~25ns engine cost) - the v2 tail
      was 11.8us of serial 610ns DMA issues on the Sync engine.
    * input DMAs: xT block 0 lands as ONE strided DMA (was 8 x 610ns
      serial issues; first matmul moved ~12.7us -> ~5us), small tensors
      (biases, masks) issue from gpsimd, weights stay on sync.
    * act table preloaded with a dummy exp during the DMA head, so the
      first real exp doesn't pay the 1.3us ACT_TABLE_LOAD.
    * all-zero V bias detected at runtime -> the 16 ones-row bias
      matmuls are compiled out (the general path stays available).
  - PSUM budget (8 banks): scores ps x2 [128,1024] (4), z accum pz x2
    [65,512] (2), fill pool x2 [128,512] (2).
"""

import sys

import numpy as np

if "/opt/trn_rl_repo" not in sys.path:
    sys.path.insert(0, "/opt/trn_rl_repo")

# Model shapes (hardcoded; kernel.py must be self-contained).
B = 2
T = 2048
D = 1024
NH = 16  # total heads
H = 4  # heads per core
DK = 64
NCORES = 8

TQ = 512  # tq chunk (matmul moving free dim)
SK = 128  # s chunk (partition dim of scoresT)

_PROGRAM_CACHE = {}

# test.py can flip these before calling kernel()
TRACE = False
TRACE_KWARGS = {}


def _build_program(t=T, d=D, reps=1, vbias=True, qkbias=True, debug=False):
    import concourse.bass as bass  # noqa: F401
    import concourse.mybir as mybir
    import concourse.tile as tile
    from concourse import bacc

    f32 = mybir.dt.float32
    bf16 = mybir.dt.bfloat16
    IDENT = mybir.ActivationFunctionType.Identity

    P = 128
    DC = d // P  # d_model chunks (contraction for projections)
    SC = t // SK  # s chunks
    TC = t // TQ  # tq chunks
    VW = H * (DK + 1)  # v block width per s-chunk: 4 heads x (64 v + 1 ones)
    m = H * DK

    nc = bacc.Bacc(
        "TRN2",
        target_bir_lowering=False,
        debug=False,
        enable_asserts=True,
        num_devices=NCORES,
    )

    xT = nc.dram_tensor("xT", [d, t], bf16, kind="ExternalInput").ap()
    wq = nc.dram_tensor("wq", [d, m], bf16, kind="ExternalInput").ap()
    wk = nc.dram_tensor("wk", [d, m], bf16, kind="ExternalInput").ap()
    wv = nc.dram_tensor("wv", [d, m], bf16, kind="ExternalInput").ap()
    wo = nc.dram_tensor("wo", [m, d], bf16, kind="ExternalInput").ap()
    qb = nc.dram_tensor("qb", [P, 2], f32, kind="ExternalInput").ap()
    kb = nc.dram_tensor("kb", [P, 2], f32, kind="ExternalInput").ap()
    vb = nc.dram_tensor("vb", [1, m], bf16, kind="ExternalInput").ap()
    masku = nc.dram_tensor("masku", [P, P], bf16, kind="ExternalInput").ap()
    ident = nc.dram_tensor("ident", [P, P], bf16, kind="ExternalInput").ap()
    out = nc.dram_tensor("out", [t, d], bf16, kind="ExternalOutput").ap()
    if debug:
        SC_ = t // SK
        VW_ = H * (DK + 1)
        dbg_xT = nc.dram_tensor("dbg_xT", [P, (d // P) * t], bf16, kind="ExternalOutput").ap()
        dbg_qT = nc.dram_tensor("dbg_qT", [P, t], bf16, kind="ExternalOutput").ap()
        dbg_kT = nc.dram_tensor("dbg_kT", [P, t], bf16, kind="ExternalOutput").ap()
        dbg_v = nc.dram_tensor("dbg_v", [P, SC_ * VW_], bf16, kind="ExternalOutput").ap()
        dbg_rcp = nc.dram_tensor("dbg_rcp", [16, TQ], f32, kind="ExternalOutput").ap()
        dbg_msk = nc.dram_tensor("dbg_msk", [P, 2 * P], bf16, kind="ExternalOutput").ap()

    with tile.TileContext(nc) as tc:
        from contextlib import ExitStack

        ctx = ExitStack()
        with ctx:
            const = ctx.enter_context(tc.tile_pool(name="const", bufs=1))

            # ---- resident SBUF tensors ----
            # xT as ONE tile so block-0 can land via a single strided DMA
            xT_all = const.tile([P, DC * t], bf16, tag="xT")
            xT_sb = [xT_all[:, c * t : (c + 1) * t] for c in range(DC)]
            wq_sb = const.tile([P, DC * m], bf16, tag="wq")
            wk_sb = const.tile([P, DC * m], bf16, tag="wk")
            wv_sb = const.tile([P, DC * m], bf16, tag="wv")
            wo_sb = const.tile([P, 2 * d], bf16, tag="wo")
            qb_sb = const.tile([P, 2], f32, tag="qb")
            kb_sb = const.tile([P, 2], f32, tag="kb")
            vb_sb = const.tile([1, m], bf16, tag="vb")
            masku_sb = const.tile([P, P], bf16, tag="masku")
            ident_sb = const.tile([P, P], bf16, tag="ident")
            ones_row = const.tile([1, P], bf16, tag="ones_row")
            v_sb = const.tile([P, SC * VW], bf16, tag="v")
            qT_sb = [
                const.tile([P, t], bf16, tag=f"qT{p}", name=f"qT{p}") for p in range(2)
            ]
            kT_sb = [
                const.tile([P, t], bf16, tag=f"kT{p}", name=f"kT{p}") for p in range(2)
            ]
            # one zT tile per (pair, tq-block): separate tiles keep the
            # O-projection of block n free of false deps on later norms
            zT_sb = [
                [
                    const.tile([P, TQ], bf16, tag=f"zT{p}_{tb}", name=f"zT{p}_{tb}")
                    for tb in range(TC)
                ]
                for p in range(2)
            ]

            # ---- input DMAs ----
            # wv + xT block0 first (v-proj of block 0 is the first PE work).
            # Bulk inputs ride TWO queues in parallel (each DMA queue tops
            # out near ~180 GB/s - half the aggregate - so every big tensor
            # splits into c-chunk halves issued from the sync and scalar
            # engines). Order on each queue = need order: wv + x block0
            # gate the first matmuls, then wq/wk, then the rest.
            HC = DC // 2

            def w_dma(dst_sb, src):
                dst = dst_sb.rearrange("p (c m) -> p c m", c=DC)
                srcr = src.rearrange("(c p) m -> p c m", p=P)
                nc.sync.dma_start(dst[:, 0:HC], srcr[:, 0:HC])
                nc.scalar.dma_start(dst[:, HC:DC], srcr[:, HC:DC])

            def x_dma(lo, hi):
                dst = xT_all.rearrange("p (c t) -> p c t", c=DC)
                srcr = xT.rearrange("(c p) t -> p c t", p=P)
                nc.sync.dma_start(dst[:, 0:HC, lo:hi], srcr[:, 0:HC, lo:hi])
                nc.scalar.dma_start(dst[:, HC:DC, lo:hi], srcr[:, HC:DC, lo:hi])

            w_dma(wv_sb, wv)
            x_dma(0, TQ)
            nc.gpsimd.dma_start(vb_sb[:], vb[:])
            nc.gpsimd.dma_start(qb_sb[:], qb[:])
            nc.gpsimd.dma_start(kb_sb[:], kb[:])
            nc.gpsimd.dma_start(masku_sb[:], masku[:])
            nc.gpsimd.dma_start(ident_sb[:], ident[:])
            w_dma(wq_sb, wq)
            w_dma(wk_sb, wk)
            if TC > 1:
                x_dma(TQ, 2 * TQ)
            wo_r = wo_sb.rearrange("p (c d2) -> p c d2", c=2)
            wo_src = wo.rearrange("(c p) d2 -> p c d2", p=P)
            nc.sync.dma_start(wo_r[:, 0:1], wo_src[:, 0:1])
            nc.scalar.dma_start(wo_r[:, 1:2], wo_src[:, 1:2])
            x_dma(2 * TQ, t)

            ones_stage = const.tile([P, 1], bf16, tag="ones_stage")
            nc.vector.memset(ones_stage[:], 1.0)
            # preload the exp act table while DMAs stream (else the first
            # real exp pays ~1.3us ACT_TABLE_LOAD on the critical path)
            warm_act = const.tile([1, 1], f32, tag="warm_act")
            nc.scalar.activation(
                warm_act[:],
                ones_stage[0:1, 0:1],
                mybir.ActivationFunctionType.Exp,
                scale=1.0,
            )
            nc.vector.tensor_copy(ones_row[:], ones_stage[0:1, :].to_broadcast([1, P]))
            # ones columns of the v blocks (col 64 of each head's 65-wide slot)
            nc.vector.tensor_copy(
                v_sb.rearrange("p (j h c) -> p j h c", h=H, c=DK + 1)[:, :, :, DK :],
                ones_stage[:, None, None, :].to_broadcast([P, SC, H, 1]),
            )

            def _phases():
                with (
                    tc.tile_pool(name="ps", bufs=2, space="PSUM") as ps_pool,
                    tc.tile_pool(name="pz", bufs=2, space="PSUM") as pz_pool,
                    tc.tile_pool(name="pfill", bufs=2, space="PSUM") as pf_pool,
                    tc.tile_pool(name="expt", bufs=6) as exp_pool,
                    tc.tile_pool(name="rcp", bufs=4) as rcp_pool,
                    tc.tile_pool(name="rbs", bufs=4) as rbs_pool,
                    tc.tile_pool(name="osb", bufs=4) as out_pool,
                ):
                    # NOTE: no PE warm-up matmuls here. They keep the HAM at
                    # 8/8 through the DMA head, but with all 8 cores running
                    # them in lockstep the CHIP power limiter kicks in (P0,
                    # PE ~2.0 GHz) and the whole run slows ~20% - measured
                    # 212us vs 176us. Power is the scarcer resource.
                    # ---------- fill closures: projections ----------
                    # each closure owns its PSUM tile's FULL lifecycle
                    # (alloc -> matmuls -> evict) so pool buffers never hold
                    # half-accumulated state across other closures.
                    def proj_closures(tp):
                        cls = []
                        sl = slice(tp * TQ, (tp + 1) * TQ)

                        def v_mm(i):
                            def go():
                                pv = pf_pool.tile([P, TQ], f32, tag="pf", name="pv")
                                for c in range(DC):
                                    nc.tensor.matmul(
                                        pv[:, 0:m],
                                        xT_sb[c][:, i * P : (i + 1) * P],
                                        wv_sb[:, c * m : (c + 1) * m],
                                        start=(c == 0),
                                        stop=(c == DC - 1) and not vbias,
                                    )
                                if vbias:
                                    nc.tensor.matmul(
                                        pv[:, 0:m],
                                        ones_row[0:1, :],
                                        vb_sb[0:1, :],
                                        start=False,
                                        stop=True,
                                    )
                                nc.vector.tensor_copy(
                                    v_sb.rearrange(
                                        "p (j h c) -> p j h c", h=H, c=DK + 1
                                    )[:, i, :, 0:DK],
                                    pv[:, 0:m].rearrange("p (h c) -> p h c", c=DK),
                                )

                            return go

                        def qk_mm(p, which):
                            w_sb = wq_sb if which == 0 else wk_sb
                            b_sb = qb_sb if which == 0 else kb_sb
                            dst = qT_sb[p] if which == 0 else kT_sb[p]

                            def go():
                                pq = pf_pool.tile([P, TQ], f32, tag="pf", name="pq")
                                for c in range(DC):
                                    nc.tensor.matmul(
                                        pq[:],
                                        w_sb[:, c * m + p * P : c * m + (p + 1) * P],
                                        xT_sb[c][:, sl],
                                        start=(c == 0),
                                        stop=(c == DC - 1),
                                    )
                                # bias + eviction fused on the Scalar engine
                                # (identity shares the exp act table, and the
                                # DVE FIFO stays clear for the norm chains)
                                nc.scalar.activation(
                                    dst[:, sl],
                                    pq[:],
                                    IDENT,
                                    bias=b_sb[:, p : p + 1],
                                    scale=1.0,
                                )

                            return go

                        for i in range(4 * tp, 4 * tp + 4):
                            cls.append(v_mm(i))
                        for p in range(2):
                            for which in range(2):
                                cls.append(qk_mm(p, which))
                        return cls

                    # ---------- fill closures: O projection ----------
                    # one closure per 128-row output chunk: both d-halves'
                    # matmuls, two casts into one [128, d] bf16 tile, ONE
                    # output DMA issued from gpsimd (software DGE).
                    def o_proj_closures(tq_c, act_evict_from=4, evict_mode=1):
                        cls = []

                        def o_mm(i, act_evict):
                            def go():
                                ic = i - 4 * tq_c
                                ot = out_pool.tile([P, d], bf16, tag="osb", name="ot")
                                for d2 in range(d // TQ):
                                    po = pf_pool.tile(
                                        [P, TQ], f32, tag="pf", name="po"
                                    )
                                    for kc in range(2):
                                        nc.tensor.matmul(
                                            po[:],
                                            zT_sb[kc][tq_c][
                                                :, ic * P : (ic + 1) * P
                                            ],
                                            wo_sb[
                                                :,
                                                kc * d
                                                + d2 * TQ : kc * d
                                                + (d2 + 1) * TQ,
                                            ],
                                            start=(kc == 0),
                                            stop=(kc == 1),
                                        )
                                    # eviction engine: 0 = DVE (mid-block),
                                    # 1 = Scalar (block tails - DVE FIFO is
                                    # busy with norm chains), 2 = split DVE/
                                    # Scalar in parallel (kernel tail, both
                                    # engines idle)
                                    dst = ot[:, d2 * TQ : (d2 + 1) * TQ]
                                    if act_evict == 1 or (
                                        act_evict == 2 and d2 == 1
                                    ):
                                        nc.scalar.activation(
                                            dst,
                                            po[:],
                                            mybir.ActivationFunctionType.Copy,
                                        )
                                    else:
                                        nc.vector.tensor_copy(dst, po[:])
                                nc.sync.dma_start(
                                    out[i * P : (i + 1) * P, :], ot[:]
                                )

                            return go

                        for n, i in enumerate(range(4 * tq_c, 4 * tq_c + 4)):
                            cls.append(o_mm(i, evict_mode if n >= act_evict_from else 0))
                        return cls

                    # ---------- attention for one tq block ----------
                    # one head-PAIR (p) at a time: per s-chunk j both heads'
                    # scores land in one 2-bank ps tile and get a single
                    # fused [128,1024] exp (fewer Act instructions).
                    def attention(tcq, fills):
                        sl = slice(tcq * TQ, (tcq + 1) * TQ)
                        nsc = 4 * tcq + 4  # s chunks in causal range
                        fill_rate = len(fills) / max(1, 2 * nsc)
                        fill_acc = [0.0]

                        def pair_loop(p):
                            pz = [
                                pz_pool.tile([DK + 1, TQ], f32, tag="pz", name="pz")
                                for _ in range(2)
                            ]

                            def scores(j):
                                ps = ps_pool.tile(
                                    [P, 2 * TQ], f32, tag="ps", name="ps"
                                )
                                rdiag = j - 4 * tcq
                                for hh in range(2):
                                    o = hh * TQ
                                    nc.tensor.matmul(
                                        ps[:, o : o + TQ],
                                        kT_sb[p][
                                            hh * DK : (hh + 1) * DK,
                                            j * SK : (j + 1) * SK,
                                        ],
                                        qT_sb[p][hh * DK : (hh + 1) * DK, sl],
                                        start=True,
                                        stop=(rdiag < 0),
                                        skip_group_check=True,
                                    )
                                    if rdiag >= 0:
                                        # causal mask of the diag 128 block:
                                        # ps += maskU^T @ I (-1e5 above diag)
                                        nc.tensor.matmul(
                                            ps[
                                                :,
                                                o
                                                + rdiag * SK : o
                                                + (rdiag + 1) * SK,
                                            ],
                                            masku_sb[:],
                                            ident_sb[:],
                                            start=False,
                                            stop=True,
                                            skip_group_check=True,
                                        )
                                et = exp_pool.tile(
                                    [P, 2 * TQ], bf16, tag="expt", name="et"
                                )
                                nc.scalar.activation(
                                    et[:],
                                    ps[:],
                                    mybir.ActivationFunctionType.Exp,
                                    scale=0.125,
                                )
                                return et

                            def zmm(j, et):
                                rdiag = j - 4 * tcq
                                for hh in range(2):
                                    o = hh * TQ
                                    h = 2 * p + hh
                                    vap = v_sb[
                                        :,
                                        j * VW
                                        + h * (DK + 1) : j * VW
                                        + (h + 1) * (DK + 1),
                                    ]
                                    if rdiag < 0:
                                        nc.tensor.matmul(
                                            pz[hh][:],
                                            vap,
                                            et[:, o : o + TQ],
                                            start=(j == 0),
                                            stop=False,
                                            skip_group_check=True,
                                        )
                                    else:
                                        nc.tensor.matmul(
                                            pz[hh][
                                                :, rdiag * SK : (rdiag + 1) * SK
                                            ],
                                            vap,
                                            et[
                                                :,
                                                o
                                                + rdiag * SK : o
                                                + (rdiag + 1) * SK,
                                            ],
                                            start=(j == 0),
                                            stop=True,
                                            skip_group_check=True,
                                        )
                                        if rdiag < 3:
                                            nc.tensor.matmul(
                                                pz[hh][:, (rdiag + 1) * SK : TQ],
                                                vap,
                                                et[
                                                    :,
                                                    o
                                                    + (rdiag + 1) * SK : o
                                                    + TQ,
                                                ],
                                                start=(j == 0),
                                                stop=False,
                                                skip_group_check=True,
                                            )

                            # software pipeline: scores run 2 j's ahead of z;
                            # fills are spread evenly over the pop slots so
                            # late units (and the norm window) stay covered
                            ets = {}
                            for u in range(nsc + 2):
                                if u < nsc:
                                    ets[u] = scores(u)
                                if u >= 2:
                                    zmm(u - 2, ets.pop(u - 2))
                                    fill_acc[0] += fill_rate
                                    while fill_acc[0] >= 1.0 and fills:
                                        fills.popleft()()
                                        fill_acc[0] -= 1.0

                            # normalization: zT = pz[0:64] * (1/denom row).
                            # reciprocal_approx_fast is ~5x cheaper than
                            # nc.vector.reciprocal and 18-bit accurate -
                            # far above the bf16 noise floor downstream.
                            for hh in range(2):
                                den = rcp_pool.tile(
                                    [1, TQ], f32, tag="den", name="den"
                                )
                                nc.vector.tensor_copy(den[:], pz[hh][DK : DK + 1, :])
                                rcp = rcp_pool.tile(
                                    [1, TQ], f32, tag="rcp", name="rcp"
                                )
                                nc.vector.reciprocal_approx_fast(rcp[:], den[:])
                                if debug:
                                    ri = 4 * tcq + 2 * p + hh
                                    nc.sync.dma_start(
                                        dbg_rcp[ri : ri + 1, :], rcp[:]
                                    )
                                rb_sb = rbs_pool.tile(
                                    [DK, TQ], f32, tag="rbs", name="rbs"
                                )
                                nc.gpsimd.partition_broadcast(rb_sb[:], rcp[:])
                                nc.vector.tensor_mul(
                                    zT_sb[p][tcq][hh * DK : (hh + 1) * DK, :],
                                    pz[hh][0:DK, :],
                                    rb_sb[:],
                                )

                        for p in range(2):
                            pair_loop(p)

                    # ---------- main schedule ----------
                    from collections import deque

                    # Fill distribution across blocks. Block tcq has
                    # 2*(4*tcq+4) fill slots; later blocks have MORE slots
                    # but no next-block projections left, so the O
                    # projections are deferred toward them (any O-proj of
                    # block b can run during any block > b). Without this,
                    # block 3's 32 units run fill-dry and pace on the Act
                    # engine's exp latency while the PE idles.
                    for cl in proj_closures(0):
                        cl()
                    for tcq in range(TC):
                        fills = deque()
                        reserve = []
                        if tcq + 1 < TC:
                            fills.extend(proj_closures(tcq + 1))
                        if tcq == TC - 2 and TC >= 3:
                            fills.extend(o_proj_closures(0))
                        if tcq == TC - 1:
                            # the last block's trailing norm chain is covered
                            # by reserve closures whose evictions ride the
                            # (then-idle) Scalar engine, not the DVE
                            if TC >= 4:
                                fills.extend(o_proj_closures(TC - 3))
                            ocls = o_proj_closures(TC - 2, act_evict_from=1)
                            fills.extend(ocls[:1])
                            reserve = ocls[1:]
                        attention(tcq, fills)
                        while fills:
                            fills.popleft()()
                        for cl in reserve:
                            cl()
                    for cl in o_proj_closures(
                        TC - 1, act_evict_from=0, evict_mode=1
                    ):
                        cl()
                    if debug:
                        nc.sync.dma_start(dbg_xT[:], xT_all[:])
                        nc.sync.dma_start(dbg_qT[:], qT_sb[0][:])
                        nc.sync.dma_start(dbg_kT[:], kT_sb[0][:])
                        nc.sync.dma_start(dbg_v[:], v_sb[:])
                        nc.sync.dma_start(dbg_msk[:, 0:P], masku_sb[:])
                        nc.sync.dma_start(dbg_msk[:, P : 2 * P], ident_sb[:])

            if reps == 1:
                _phases()
            else:
                with tc.For_i(0, reps, 1):
                    _phases()

    nc.compile()
    return nc


def _get_program(t=T, d=D, reps=1, vbias=True, qkbias=True, debug=False):
    key = (t, d, reps, vbias, qkbias, debug)
    if key not in _PROGRAM_CACHE:
        _PROGRAM_CACHE[key] = _build_program(t, d, reps, vbias, qkbias, debug)
    return _PROGRAM_CACHE[key]


def _bf16(a):
    import ml_dtypes

    return np.ascontiguousarray(a.astype(ml_dtypes.bfloat16))


def _make_masks():
    import ml_dtypes

    i = np.arange(SK)[:, None]
    j = np.arange(SK)[None, :]
    # masku[k, m] = -1e5 where m > k (strictly upper): M = masku^T @ I gives
    # M[s, t] = masku[t, s] = -1e5 where s > t (key after query -> masked)
    masku = np.where(j > i, np.float32(-100000.0), np.float32(0.0))
    ident = np.eye(SK, dtype=np.float32)
    return (
        np.ascontiguousarray(masku.astype(ml_dtypes.bfloat16)),
        np.ascontiguousarray(ident.astype(ml_dtypes.bfloat16)),
    )


def _core_inputs(x, Qs, Qbs, Ks, Kbs, Vs, Vbs, O, b, g, masku, ident):
    hs = slice(H * g, H * (g + 1))
    xT_b = _bf16(np.ascontiguousarray(x[b].T))  # [D, T]
    wq_g = _bf16(Qs[hs].transpose(1, 0, 2).reshape(D, H * DK))
    wk_g = _bf16(Ks[hs].transpose(1, 0, 2).reshape(D, H * DK))
    wv_g = _bf16(Vs[hs].transpose(1, 0, 2).reshape(D, H * DK))
    wo_g = _bf16(O[hs].reshape(H * DK, D))
    qb_flat = Qbs[hs].reshape(H * DK)
    kb_flat = Kbs[hs].reshape(H * DK)
    qb_g = np.ascontiguousarray(np.stack([qb_flat[0:128], qb_flat[128:256]], axis=1))
    kb_g = np.ascontiguousarray(np.stack([kb_flat[0:128], kb_flat[128:256]], axis=1))
    vb_g = _bf16(Vbs[hs].reshape(1, H * DK))
    return {
        "xT": xT_b,
        "wq": wq_g,
        "wk": wk_g,
        "wv": wv_g,
        "wo": wo_g,
        "qb": qb_g,
        "kb": kb_g,
        "vb": vb_g,
        "masku": masku,
        "ident": ident,
    }


def _build_in_maps(x, Qs, Qbs, Ks, Kbs, Vs, Vbs, O_):
    masku, ident = _make_masks()
    in_maps = []
    for core in range(NCORES):
        b, g = divmod(core, NH // H)
        in_maps.append(
            _core_inputs(x, Qs, Qbs, Ks, Kbs, Vs, Vbs, O_, b, g, masku, ident)
        )
    return in_maps


def kernel(normalized_resid_pre, Qs, Qbs, Ks, Kbs, Vs, Vbs, O, Ob):
    from concourse.bass_utils import run_bass_kernel_spmd

    x = np.asarray(normalized_resid_pre, dtype=np.float32)
    Qs, Qbs = np.asarray(Qs, np.float32), np.asarray(Qbs, np.float32)
    Ks, Kbs = np.asarray(Ks, np.float32), np.asarray(Kbs, np.float32)
    Vs, Vbs = np.asarray(Vs, np.float32), np.asarray(Vbs, np.float32)
    O_, Ob = np.asarray(O, np.float32), np.asarray(Ob, np.float32)

    vbias = bool(np.any(Vbs != 0.0))
    qkbias = bool(np.any(Qbs != 0.0) or np.any(Kbs != 0.0))
    nc = _get_program(vbias=vbias, qkbias=qkbias)
    in_maps = _build_in_maps(x, Qs, Qbs, Ks, Kbs, Vs, Vbs, O_)

    res = run_bass_kernel_spmd(
        nc, in_maps, core_ids=list(range(NCORES)), trace=TRACE, **TRACE_KWARGS
    )
    kernel.last_results = res

    out = np.zeros((B, T, D), dtype=np.float32)
    for core in range(NCORES):
        b, g = divmod(core, NH // H)
        out[b] += np.asarray(res.results[core]["out"], dtype=np.float32)
    out += Ob[None, None, :]
    return out
